# revision 7
# baseline (speedup 1.0000x reference)
"""Single-head attention (B=4, N=2048, D=1024), scores scaled by 10.

Sharding: 8 cores = (batch, query-half). Core 2b+h owns queries
[1024h:1024(h+1)] of batch b. K/V projections are computed for the OWN
half only and exchanged with the pair core (2b ^ 1) via an on-chip
AllGather, halving the projection FLOPs vs recomputing the full
sequence per core. Key order is global [h0|h1] (rank order) on every
core, so the SPMD program is identical across cores.

Numerics: everything runs single-pass fp16 (fp32 PSUM accumulate).
Measured end-to-end rel err ~6e-3 vs the fp32 reference (2e-2 gate).
The per-query max subtraction cancels exactly in the softmax
normalization, so max precision only guards overflow.

Schedule: inputs are host-pre-tiled so every load is a contiguous
>=256KB DMA, split across the two HWDGE rings (sync + scalar) and into
per-use tiles so the first matmul fires at ~7us. Phase order K -> V ->
Q gets the K AllGather launched as early as possible; both collectives
ride gpsimd (serial, ~35us each), K readback rides the scalar ring, vf
readback queues behind the V collective on gpsimd. Phase B runs a
2-deep software pipeline: PV(c) is emitted after QKh2(c+1) AND
QKh1(c+2), which hides the softmax chain AND tolerates the V exchange
finishing ~40us into phase B.
"""

import numpy as np

B, SEQ, D = 4, 2048, 1024
NQ = 1024          # queries per core (= keys computed per core)
QCH = 256          # attention q-chunk
NCH = NQ // QCH
NCORES = 8
DT = D // 128      # 8 d-tiles
ET = D // 128      # 8 e-tiles
KT = SEQ // 128    # 16 k-tiles
HKT = KT // 2      # 8 own-half k-tiles

_BUILT = {}


def _build():
    if "nc" in _BUILT:
        return _BUILT["nc"]
    from contextlib import ExitStack

    import concourse.bass as bass  # noqa: F401
    import concourse.mybir as mybir
    import concourse.tile as tile
    from concourse import bacc

    dt = mybir.dt
    F32, F16 = dt.float32, dt.float16
    BF = dt.bfloat16
    AL = mybir.AluOpType
    EXP = mybir.ActivationFunctionType.Exp
    GROUPS = [[2 * i, 2 * i + 1] for i in range(NCORES // 2)]

    nc = bacc.Bacc("TRN2", target_bir_lowering=False, debug=False)

    # host-pre-tiled inputs: each leading index is one contiguous DMA with
    # >=1KB per-partition lines
    xf_d = nc.dram_tensor("xf", [2, 128, DT, 512], F16, kind="ExternalInput")
    wq_d = nc.dram_tensor("wq", [ET, 128, DT, 128], F16, kind="ExternalInput")
    wk_d = nc.dram_tensor("wk", [ET, 128, DT, 128], F16, kind="ExternalInput")
    wv_d = nc.dram_tensor("wv", [2, 128, DT, 512], F16, kind="ExternalInput")
    ot_d = nc.dram_tensor("ot", [D, NQ], F16, kind="ExternalOutput")

    xf_r = xf_d.ap()
    wq_r = wq_d.ap()
    wk_r = wk_d.ap()
    wv_r = wv_d.ap()
    ot_r = ot_d.ap().rearrange("(t p) q -> p t q", p=128)

    with tile.TileContext(nc) as tc, ExitStack() as ctx:
        qk_pool = ctx.enter_context(tc.tile_pool(name="qk", bufs=1))
        qt = qk_pool.tile([128, ET, NQ], F16, tag="qt")
        kt = qk_pool.tile([128, ET, SEQ], F16, tag="kt")
        v_pool = ctx.enter_context(tc.tile_pool(name="vp", bufs=1))
        vf = v_pool.tile([128, KT, D], F16, tag="vf")

        const_pool = ctx.enter_context(tc.tile_pool(name="const", bufs=1))
        ones16 = const_pool.tile([128, 1], F16, tag="ones16")
        ten32 = const_pool.tile([1, 128], F32, tag="ten32")
        one32 = const_pool.tile([1, 128], F32, tag="one32")

        dram = ctx.enter_context(tc.tile_pool(name="dram", bufs=1, space="DRAM"))
        # own-half K^T rows; AllGather output has rank blocks [r0, r1]
        k_in = dram.tile([D, NQ], F16, tag="k_in")
        k_out = dram.tile([2 * D, NQ], F16, tag="k_out")
        v_in = dram.tile([NQ, D], F16, tag="v_in")
        v_out = dram.tile([SEQ, D], F16, tag="v_out")
        warm_in = dram.tile([16, 16], BF, tag="warm_in")
        warm_out = dram.tile([32, 16], BF, tag="warm_out")

        # tiny warmup collective at t=0: pays the ncfw channel-setup latency
        # before the real exchanges need it
        warm_sb = const_pool.tile([16, 16], BF, tag="warm_sb")
        nc.vector.memset(warm_sb[:], 0.0)
        nc.sync.dma_start(warm_in[:], warm_sb[:])
        nc.gpsimd.collective_compute(
            "AllGather",
            AL.bypass,
            replica_groups=GROUPS,
            ins=[warm_in[:]],
            outs=[warm_out[:]],
        )
        nc.vector.memset(ones16[:], 1.0)
        nc.vector.memset(ten32[:], 10.0)
        nc.vector.memset(one32[:], 1.0)

        with (
            tc.tile_pool(name="xspan", bufs=1) as xspan,
            tc.tile_pool(name="wstr", bufs=1) as wpool,
            tc.tile_pool(name="kev", bufs=3) as kevpool,
            tc.tile_pool(name="psA", bufs=4, space="PSUM") as psA,
        ):
            xf_t = [
                xspan.tile([128, DT, 512], F16, tag=f"xf{i}", name=f"xf{i}")
                for i in range(2)
            ]
            wk_t = [
                wpool.tile([128, DT, 128], F16, tag=f"wk{e}", name=f"wk{e}")
                for e in range(ET)
            ]
            wq_t = [
                wpool.tile([128, DT, 128], F16, tag=f"wq{e}", name=f"wq{e}")
                for e in range(ET)
            ]
            wv_t = [
                wpool.tile([128, DT, 512], F16, tag=f"wv{i}", name=f"wv{i}")
                for i in range(2)
            ]
            # input DMAs up front, split across the two HWDGE rings (sync ->
            # qSPDynamicHW, scalar -> qActDynamicHW), ordered by first use
            nc.sync.dma_start(wk_t[0][:], wk_r[0])
            nc.sync.dma_start(xf_t[0][:], xf_r[0])
            for e in range(1, ET):
                nc.sync.dma_start(wk_t[e][:], wk_r[e])
            nc.sync.dma_start(xf_t[1][:], xf_r[1])
            for i in range(2):
                nc.scalar.dma_start(wv_t[i][:], wv_r[i])
            for e in range(ET):
                nc.scalar.dma_start(wq_t[e][:], wq_r[e])

            # ---------------- Phase K: own-half K^T projection -------------
            for chn in range(2):
                n0 = 512 * chn
                for et in range(ET):
                    e0 = 128 * et
                    ps = psA.tile([128, 512], F32, tag="psA")
                    for dti in range(DT):
                        nc.tensor.matmul(
                            ps[:],
                            wk_t[et][:, dti, :],
                            xf_t[chn][:, dti, :],
                            start=(dti == 0),
                            stop=(dti == DT - 1),
                        )
                    kev = kevpool.tile([128, 512], F16, tag="kev")
                    nc.vector.tensor_copy(kev[:], ps[:])
                    nc.scalar.dma_start(
                        k_in[e0 : e0 + 128, n0 : n0 + 512], kev[:]
                    )

            # pair AllGather of K halves
            nc.gpsimd.collective_compute(
                "AllGather",
                AL.bypass,
                replica_groups=GROUPS,
                ins=[k_in[:]],
                outs=[k_out[:]],
            )
            # K readback on the scalar ring: keeps gpsimd free to launch the
            # V collective the moment the K collective completes
            k_out_r = k_out[:].rearrange("(b t p) n -> b p t n", p=128, t=ET)
            for h in range(2):
                nc.scalar.dma_start(
                    kt[:, :, NQ * h : NQ * (h + 1)], k_out_r[h, :, :, :]
                )

            # ------------- Phase V: own-half V projection ------------------
            for ec in range(2):
                e0 = 512 * ec
                for kti in range(HKT):
                    k0 = 128 * (kti % 4)
                    ps = psA.tile([128, 512], F32, tag="psA")
                    for dti in range(DT):
                        nc.tensor.matmul(
                            ps[:],
                            xf_t[kti // 4][:, dti, k0 : k0 + 128],
                            wv_t[ec][:, dti, :],
                            start=(dti == 0),
                            stop=(dti == DT - 1),
                        )
                    vev = kevpool.tile([128, 512], F16, tag="vev")
                    nc.vector.tensor_copy(vev[:], ps[:])
                    nc.scalar.dma_start(
                        v_in[128 * kti : 128 * kti + 128, e0 : e0 + 512], vev[:]
                    )

            nc.gpsimd.collective_compute(
                "AllGather",
                AL.bypass,
                replica_groups=GROUPS,
                ins=[v_in[:]],
                outs=[v_out[:]],
            )
            # vf readback queues right behind the V collective on gpsimd
            v_out_r = v_out[:].rearrange("(b t p) e -> b p t e", p=128, t=HKT)
            for h in range(2):
                nc.gpsimd.dma_start(
                    vf[:, HKT * h : HKT * (h + 1), :], v_out_r[h, :, :, :]
                )

            # ------------- Phase Q: own-half Q^T projection ----------------
            for et in range(ET):
                for chn in range(2):
                    n0 = 512 * chn
                    ps = psA.tile([128, 512], F32, tag="psA")
                    for dti in range(DT):
                        nc.tensor.matmul(
                            ps[:],
                            wq_t[et][:, dti, :],
                            xf_t[chn][:, dti, :],
                            start=(dti == 0),
                            stop=(dti == DT - 1),
                        )
                    nc.vector.tensor_copy(qt[:, et, n0 : n0 + 512], ps[:])

        # ---------------- Phase B: attention, q-chunked, pipelined --------
        with (
            tc.tile_pool(name="stp", bufs=3) as stpool,
            tc.tile_pool(name="pp", bufs=2) as ppool,
            tc.tile_pool(name="tree", bufs=3) as treepool,
            tc.tile_pool(name="aux", bufs=2) as auxpool,
            tc.tile_pool(name="osb", bufs=3) as outpool,
            tc.tile_pool(name="psS", bufs=3, space="PSUM") as psS,
            tc.tile_pool(name="psO", bufs=2, space="PSUM") as psO,
            tc.tile_pool(name="psX", bufs=2, space="PSUM") as psX,
            tc.tile_pool(name="psR", bufs=1, space="PSUM") as psR,
        ):
            st_ap = {}
            t8_ap = {}
            p_ap = {}
            maxb_ap = {}
            m1_ap = {}

            def emit_qk_half(c, half):
                """Scores for k-tiles [8h, 8h+8) of chunk c, with the DVE
                max-tree levels interleaved behind the PSUM copies."""
                q0 = QCH * c
                if half == 0:
                    st_ap[c] = stpool.tile([128, KT, QCH], F32, tag="st", name="st")
                    t8_ap[c] = treepool.tile([128, 8, QCH], F32, tag="t8", name="t8")
                st, t8 = st_ap[c], t8_ap[c]
                for kti in range(8 * half, 8 * half + 8):
                    k0 = 128 * kti
                    ps = psS.tile([128, QCH], F32, tag="psS")
                    for et in range(ET):
                        nc.tensor.matmul(
                            ps[:],
                            kt[:, et, k0 : k0 + 128],
                            qt[:, et, q0 : q0 + QCH],
                            start=(et == 0),
                            stop=(et == ET - 1),
                        )
                    nc.vector.tensor_copy(st[:, kti, :], ps[:])
                    # interleave the max tree: level-0 after each odd tile,
                    # higher levels as their inputs complete
                    if kti % 2 == 1:
                        j = kti // 2
                        nc.vector.tensor_max(
                            t8[:, j, :], st[:, kti - 1, :], st[:, kti, :]
                        )
                        if j % 2 == 1:
                            nc.vector.tensor_max(
                                t8[:, j - 1, :], t8[:, j - 1, :], t8[:, j, :]
                            )
                        if j == 3:
                            nc.vector.tensor_max(
                                t8[:, 0, :], t8[:, 0, :], t8[:, 2, :]
                            )
                        if j == 7:
                            nc.vector.tensor_max(
                                t8[:, 4, :], t8[:, 4, :], t8[:, 6, :]
                            )

            def emit_tree_finish(c):
                """Final fold 128 partitions -> one [1, QCH] max row."""
                t8 = t8_ap[c]
                nc.vector.tensor_max(t8[:, 0, :], t8[:, 0, :], t8[:, 4, :])
                fold4 = treepool.tile([32, 4, QCH], F32, tag="fold4")
                for a in range(4):
                    nc.sync.dma_start(
                        fold4[:, a, :], t8[32 * a : 32 * (a + 1), 0, :]
                    )
                nc.vector.tensor_max(fold4[:, 0, :], fold4[:, 0, :], fold4[:, 1, :])
                nc.vector.tensor_max(fold4[:, 2, :], fold4[:, 2, :], fold4[:, 3, :])
                nc.vector.tensor_max(fold4[:, 0, :], fold4[:, 0, :], fold4[:, 2, :])
                t32t = treepool.tile([32, QCH], F32, tag="t32t")
                nc.vector.transpose(t32t[:], fold4[:, 0, :])
                mx32 = treepool.tile([32, 32], F32, tag="mx32")
                nc.vector.memset(mx32[:], 0.0)
                nc.vector.reduce_max(
                    mx32[:, 0 : QCH // 32],
                    t32t[:].rearrange("p (j c) -> p j c", c=32),
                    axis=mybir.AxisListType.X,
                )
                mx32t = treepool.tile([32, 32], F32, tag="mx32t")
                nc.vector.transpose(mx32t[:], mx32[:])
                m1row = treepool.tile([1, QCH], F32, tag="m1row")
                nc.sync.dma_start(m1row[:], mx32t[0 : QCH // 32, :])
                m1_ap[c] = m1row

            def emit_maxb(c):
                """Broadcast 10*max across partitions via rank-1 matmul."""
                maxb_ps = psX.tile([128, QCH], F32, tag="bcast")
                nc.tensor.matmul(
                    maxb_ps[:], ten32[:], m1_ap[c][:], start=True, stop=True
                )
                maxb = auxpool.tile([128, QCH], F32, tag="maxb")
                nc.vector.tensor_copy(maxb[:], maxb_ps[:])
                maxb_ap[c] = maxb

            def emit_stt_exp(c):
                """st = 10*st - maxb, then P = exp(st) in fp16 (batched)."""
                st, maxb = st_ap[c], maxb_ap[c]
                p_ap[c] = ppool.tile([128, KT, QCH], F16, tag="p", name="p")
                for kti in range(KT):
                    nc.vector.scalar_tensor_tensor(
                        st[:, kti, :],
                        st[:, kti, :],
                        10.0,
                        maxb[:],
                        op0=AL.mult,
                        op1=AL.subtract,
                    )
                    if kti % 4 == 3:
                        nc.scalar.activation(
                            p_ap[c][:, kti - 3 : kti + 1, :],
                            st[:, kti - 3 : kti + 1, :],
                            EXP,
                        )

            def emit_sum_recb(c):
                """Key-sums of P via ones-matmul, 1/sum, broadcast."""
                sum_ps = psR.tile([1, QCH], F32, tag="sum")
                for kti in range(KT):
                    nc.tensor.matmul(
                        sum_ps[:],
                        ones16[:],
                        p_ap[c][:, kti, :],
                        start=(kti == 0),
                        stop=(kti == KT - 1),
                    )
                recrow = treepool.tile([1, QCH], F32, tag="recrow")
                nc.vector.reciprocal(recrow[:], sum_ps[:])
                recb_ps = psX.tile([128, QCH], F32, tag="bcast")
                nc.tensor.matmul(
                    recb_ps[:], one32[:], recrow[:], start=True, stop=True
                )
                recb = auxpool.tile([128, QCH], F32, tag="recb")
                nc.vector.tensor_copy(recb[:], recb_ps[:])
                return recb

            def emit_pv(c, recb):
                """O^T[d, q] = V^T P scaled by 1/sum, written out as fp16."""
                q0 = QCH * c
                for dti in range(DT):
                    d0 = 128 * dti
                    ops = psO.tile([128, QCH], F32, tag="psO")
                    for kti in range(KT):
                        nc.tensor.matmul(
                            ops[:],
                            vf[:, kti, d0 : d0 + 128],
                            p_ap[c][:, kti, :],
                            start=(kti == 0),
                            stop=(kti == KT - 1),
                        )
                    osb = outpool.tile([128, QCH], F16, tag="osb")
                    nc.vector.scalar_tensor_tensor(
                        osb[:], ops[:], 1.0, recb[:], op0=AL.mult, op1=AL.mult
                    )
                    nc.sync.dma_start(ot_r[:, dti, q0 : q0 + QCH], osb[:])

            emit_qk_half(0, 0)
            emit_qk_half(0, 1)
            emit_qk_half(1, 0)
            for c in range(NCH):
                emit_tree_finish(c)
                emit_maxb(c)
                emit_stt_exp(c)
                if c + 1 < NCH:
                    emit_qk_half(c + 1, 1)
                recb = emit_sum_recb(c)
                if c + 2 < NCH:
                    emit_qk_half(c + 2, 0)
                emit_pv(c, recb)

    nc.compile()
    _BUILT["nc"] = nc
    return nc


def _tile_rows(a, blocks):
    """[D_in, cols] -> [blocks, 128, DT, cols/blocks]: row d, block j lands
    at [j, d % 128, d // 128, :]."""
    cols = a.shape[1] // blocks
    out = np.empty((blocks, 128, DT, cols), a.dtype)
    t = a.reshape(DT, 128, a.shape[1]).transpose(1, 0, 2)
    for j in range(blocks):
        out[j] = t[:, :, j * cols : (j + 1) * cols]
    return np.ascontiguousarray(out)


def _prep_inputs(x, q_w, k_w, v_w):
    wq = _tile_rows(q_w.T.astype(np.float16), ET)
    wk = _tile_rows(k_w.T.astype(np.float16), ET)
    wv = _tile_rows(v_w.T.astype(np.float16), 2)

    in_maps = []
    for core in range(NCORES):
        b, h = divmod(core, 2)
        xt = _tile_rows(
            np.asarray(x[b, NQ * h : NQ * (h + 1)]).T.astype(np.float16), 2
        )
        in_maps.append({"xf": xt, "wq": wq, "wk": wk, "wv": wv})
    return in_maps


def run(x, q_w, k_w, v_w, trace=False):
    from concourse.bass_utils import run_bass_kernel_spmd

    nc = _build()
    in_maps = _prep_inputs(x, q_w, k_w, v_w)
    res = run_bass_kernel_spmd(nc, in_maps, list(range(NCORES)), trace=trace)
    out = np.empty((B, SEQ, D), np.float32)
    for core in range(NCORES):
        b, h = divmod(core, 2)
        out[b, NQ * h : NQ * (h + 1)] = res.results[core]["ot"].T.astype(np.float32)
    return out, res


def kernel(x, q_w, k_w, v_w):
    x = np.asarray(x, np.float32)
    q_w = np.asarray(q_w, np.float32)
    k_w = np.asarray(k_w, np.float32)
    v_w = np.asarray(v_w, np.float32)
    out, _ = run(x, q_w, k_w, v_w, trace=False)
    return out


# revision 8
# speedup vs baseline: 1.2805x; 1.2805x over previous
"""Single-head attention (B=4, N=2048, D=1024), scores scaled by 10.

Sharding: 8 cores = (batch, query-half). Core 2b+h owns queries
[1024h:1024(h+1)] of batch b.

Algebraic restructuring: scores = Q K^T = x (Wq^T Wk) x^T. The d x d
matrix A = Wq^T Wk is precomputed on the host, the device computes
XA = x_own @ A (cost of one projection) and uses the raw FULL x as the
key matrix — the K projection, the K pair-exchange, and its readback
all disappear. V is still projected for the OWN half only and
exchanged with the pair core (2b ^ 1) via an AllGather (the only
collective, launched early with a ~100us hiding window).

Numerics: single-pass fp16 matmuls (fp32 PSUM accumulate); measured
end-to-end rel err ~4.6e-3 vs the fp32 reference (2e-2 gate). The
per-query max subtraction cancels exactly in the softmax
normalization, so max precision only guards overflow.

Schedule: inputs host-pre-tiled into contiguous >=256KB per-use DMAs,
split across the two HWDGE rings. Phase V first (feeds the AllGather),
then XA; phase B starts ~60us in. Phase B runs a 2-deep software
pipeline: PV(c) is emitted after QKh2(c+1) and QKh1(c+2), hiding the
DVE max-tree / ACT exp chain and tolerating a late V exchange.
"""

import numpy as np

B, SEQ, D = 4, 2048, 1024
NQ = 1024          # queries per core
QCH = 256          # attention q-chunk
NCH = NQ // QCH
NCORES = 8
DT = D // 128      # 8 d-tiles
ET = D // 128      # 8 e-tiles
KT = SEQ // 128    # 16 k-tiles
HKT = KT // 2      # 8 own-half k-tiles

_BUILT = {}


def _build():
    if "nc" in _BUILT:
        return _BUILT["nc"]
    from contextlib import ExitStack

    import concourse.bass as bass  # noqa: F401
    import concourse.mybir as mybir
    import concourse.tile as tile
    from concourse import bacc

    dt = mybir.dt
    F32, F16 = dt.float32, dt.float16
    BF = dt.bfloat16
    AL = mybir.AluOpType
    EXP = mybir.ActivationFunctionType.Exp
    GROUPS = [[2 * i, 2 * i + 1] for i in range(NCORES // 2)]

    nc = bacc.Bacc("TRN2", target_bir_lowering=False, debug=False)

    # host-pre-tiled inputs: each leading index is one contiguous DMA
    xq_d = nc.dram_tensor("xq", [2, 128, DT, 512], F16, kind="ExternalInput")
    xk_d = nc.dram_tensor("xk", [4, 128, DT, 512], F16, kind="ExternalInput")
    am_d = nc.dram_tensor("am", [ET, 128, DT, 128], F16, kind="ExternalInput")
    wv_d = nc.dram_tensor("wv", [2, 128, DT, 512], F16, kind="ExternalInput")
    ot_d = nc.dram_tensor("ot", [D, NQ], F16, kind="ExternalOutput")

    xq_r = xq_d.ap()
    xk_r = xk_d.ap()
    am_r = am_d.ap()
    wv_r = wv_d.ap()
    ot_r = ot_d.ap().rearrange("(t p) q -> p t q", p=128)

    with tile.TileContext(nc) as tc, ExitStack() as ctx:
        qk_pool = ctx.enter_context(tc.tile_pool(name="qk", bufs=1))
        # XA^T [d-part, d-tile, own queries]
        xat = qk_pool.tile([128, ET, NQ], F16, tag="xat")
        # full x^T (the key matrix), 4 column blocks of 512 keys
        xk_t = [
            qk_pool.tile([128, DT, 512], F16, tag=f"xk{i}", name=f"xk{i}")
            for i in range(4)
        ]
        v_pool = ctx.enter_context(tc.tile_pool(name="vp", bufs=1))
        vf = v_pool.tile([128, KT, D], F16, tag="vf")

        const_pool = ctx.enter_context(tc.tile_pool(name="const", bufs=1))
        ones16 = const_pool.tile([128, 1], F16, tag="ones16")
        ten32 = const_pool.tile([1, 128], F32, tag="ten32")
        one32 = const_pool.tile([1, 128], F32, tag="one32")

        dram = ctx.enter_context(tc.tile_pool(name="dram", bufs=1, space="DRAM"))
        v_in = dram.tile([NQ, D], F16, tag="v_in")
        v_out = dram.tile([SEQ, D], F16, tag="v_out")
        warm_in = dram.tile([16, 16], BF, tag="warm_in")
        warm_out = dram.tile([32, 16], BF, tag="warm_out")

        # tiny warmup collective at t=0: pays the ncfw channel-setup latency
        # before the real V exchange needs it
        warm_sb = const_pool.tile([16, 16], BF, tag="warm_sb")
        nc.vector.memset(warm_sb[:], 0.0)
        nc.sync.dma_start(warm_in[:], warm_sb[:])
        nc.gpsimd.collective_compute(
            "AllGather",
            AL.bypass,
            replica_groups=GROUPS,
            ins=[warm_in[:]],
            outs=[warm_out[:]],
        )
        nc.vector.memset(ones16[:], 1.0)
        nc.vector.memset(ten32[:], 10.0)
        nc.vector.memset(one32[:], 1.0)

        with (
            tc.tile_pool(name="xspan", bufs=1) as xspan,
            tc.tile_pool(name="wstr", bufs=1) as wpool,
            tc.tile_pool(name="kev", bufs=3) as kevpool,
            tc.tile_pool(name="psA", bufs=4, space="PSUM") as psA,
        ):
            xq_t = [
                xspan.tile([128, DT, 512], F16, tag=f"xq{i}", name=f"xq{i}")
                for i in range(2)
            ]
            am_t = [
                wpool.tile([128, DT, 128], F16, tag=f"am{e}", name=f"am{e}")
                for e in range(ET)
            ]
            wv_t = [
                wpool.tile([128, DT, 512], F16, tag=f"wv{i}", name=f"wv{i}")
                for i in range(2)
            ]
            # input DMAs up front, split across the two HWDGE rings (sync ->
            # qSPDynamicHW, scalar -> qActDynamicHW), ordered by first use
            nc.sync.dma_start(wv_t[0][:], wv_r[0])
            nc.sync.dma_start(xq_t[0][:], xq_r[0])
            nc.sync.dma_start(xq_t[1][:], xq_r[1])
            nc.sync.dma_start(wv_t[1][:], wv_r[1])
            for i in range(4):
                nc.sync.dma_start(xk_t[i][:], xk_r[i])
            for e in range(ET):
                nc.scalar.dma_start(am_t[e][:], am_r[e])

            # ------------- Phase V: own-half V projection ------------------
            for ec in range(2):
                e0 = 512 * ec
                for kti in range(HKT):
                    k0 = 128 * (kti % 4)
                    ps = psA.tile([128, 512], F32, tag="psA")
                    for dti in range(DT):
                        nc.tensor.matmul(
                            ps[:],
                            xq_t[kti // 4][:, dti, k0 : k0 + 128],
                            wv_t[ec][:, dti, :],
                            start=(dti == 0),
                            stop=(dti == DT - 1),
                        )
                    vev = kevpool.tile([128, 512], F16, tag="vev")
                    nc.vector.tensor_copy(vev[:], ps[:])
                    nc.scalar.dma_start(
                        v_in[128 * kti : 128 * kti + 128, e0 : e0 + 512], vev[:]
                    )

            # pair AllGather of V halves; vf readback queues right behind it
            # on gpsimd
            nc.gpsimd.collective_compute(
                "AllGather",
                AL.bypass,
                replica_groups=GROUPS,
                ins=[v_in[:]],
                outs=[v_out[:]],
            )
            v_out_r = v_out[:].rearrange("(b t p) e -> b p t e", p=128, t=HKT)
            for h in range(2):
                nc.gpsimd.dma_start(
                    vf[:, HKT * h : HKT * (h + 1), :], v_out_r[h, :, :, :]
                )

            # ------------- Phase XA: own-half x @ A (transposed) -----------
            for et in range(ET):
                for chn in range(2):
                    n0 = 512 * chn
                    ps = psA.tile([128, 512], F32, tag="psA")
                    for dti in range(DT):
                        nc.tensor.matmul(
                            ps[:],
                            am_t[et][:, dti, :],
                            xq_t[chn][:, dti, :],
                            start=(dti == 0),
                            stop=(dti == DT - 1),
                        )
                    nc.vector.tensor_copy(xat[:, et, n0 : n0 + 512], ps[:])

        # ---------------- Phase B: attention, q-chunked, pipelined --------
        with (
            tc.tile_pool(name="stp", bufs=3) as stpool,
            tc.tile_pool(name="pp", bufs=2) as ppool,
            tc.tile_pool(name="tree", bufs=3) as treepool,
            tc.tile_pool(name="aux", bufs=2) as auxpool,
            tc.tile_pool(name="osb", bufs=3) as outpool,
            tc.tile_pool(name="psS", bufs=3, space="PSUM") as psS,
            tc.tile_pool(name="psO", bufs=2, space="PSUM") as psO,
            tc.tile_pool(name="psX", bufs=2, space="PSUM") as psX,
            tc.tile_pool(name="psR", bufs=1, space="PSUM") as psR,
        ):
            st_ap = {}
            t8_ap = {}
            p_ap = {}
            maxb_ap = {}
            m1_ap = {}

            def emit_qk_half(c, half):
                """Scores for k-tiles [8h, 8h+8) of chunk c, with the DVE
                max-tree levels interleaved behind the PSUM copies."""
                q0 = QCH * c
                if half == 0:
                    st_ap[c] = stpool.tile([128, KT, QCH], F32, tag="st", name="st")
                    t8_ap[c] = treepool.tile([128, 8, QCH], F32, tag="t8", name="t8")
                st, t8 = st_ap[c], t8_ap[c]
                for kti in range(8 * half, 8 * half + 8):
                    k0 = 128 * (kti % 4)
                    ps = psS.tile([128, QCH], F32, tag="psS")
                    for dti in range(DT):
                        nc.tensor.matmul(
                            ps[:],
                            xk_t[kti // 4][:, dti, k0 : k0 + 128],
                            xat[:, dti, q0 : q0 + QCH],
                            start=(dti == 0),
                            stop=(dti == DT - 1),
                        )
                    nc.vector.tensor_copy(st[:, kti, :], ps[:])
                    # interleave the max tree: level-0 after each odd tile,
                    # higher levels as their inputs complete
                    if kti % 2 == 1:
                        j = kti // 2
                        nc.vector.tensor_max(
                            t8[:, j, :], st[:, kti - 1, :], st[:, kti, :]
                        )
                        if j % 2 == 1:
                            nc.vector.tensor_max(
                                t8[:, j - 1, :], t8[:, j - 1, :], t8[:, j, :]
                            )
                        if j == 3:
                            nc.vector.tensor_max(
                                t8[:, 0, :], t8[:, 0, :], t8[:, 2, :]
                            )
                        if j == 7:
                            nc.vector.tensor_max(
                                t8[:, 4, :], t8[:, 4, :], t8[:, 6, :]
                            )

            def emit_tree_finish(c):
                """Final fold 128 partitions -> one [1, QCH] max row."""
                t8 = t8_ap[c]
                nc.vector.tensor_max(t8[:, 0, :], t8[:, 0, :], t8[:, 4, :])
                fold4 = treepool.tile([32, 4, QCH], F32, tag="fold4")
                for a in range(4):
                    nc.sync.dma_start(
                        fold4[:, a, :], t8[32 * a : 32 * (a + 1), 0, :]
                    )
                nc.vector.tensor_max(fold4[:, 0, :], fold4[:, 0, :], fold4[:, 1, :])
                nc.vector.tensor_max(fold4[:, 2, :], fold4[:, 2, :], fold4[:, 3, :])
                nc.vector.tensor_max(fold4[:, 0, :], fold4[:, 0, :], fold4[:, 2, :])
                t32t = treepool.tile([32, QCH], F32, tag="t32t")
                nc.vector.transpose(t32t[:], fold4[:, 0, :])
                mx32 = treepool.tile([32, 32], F32, tag="mx32")
                nc.vector.memset(mx32[:], 0.0)
                nc.vector.reduce_max(
                    mx32[:, 0 : QCH // 32],
                    t32t[:].rearrange("p (j c) -> p j c", c=32),
                    axis=mybir.AxisListType.X,
                )
                mx32t = treepool.tile([32, 32], F32, tag="mx32t")
                nc.vector.transpose(mx32t[:], mx32[:])
                m1row = treepool.tile([1, QCH], F32, tag="m1row")
                nc.sync.dma_start(m1row[:], mx32t[0 : QCH // 32, :])
                m1_ap[c] = m1row

            def emit_maxb(c):
                """Broadcast 10*max across partitions via rank-1 matmul."""
                maxb_ps = psX.tile([128, QCH], F32, tag="bcast")
                nc.tensor.matmul(
                    maxb_ps[:], ten32[:], m1_ap[c][:], start=True, stop=True
                )
                maxb = auxpool.tile([128, QCH], F32, tag="maxb")
                nc.vector.tensor_copy(maxb[:], maxb_ps[:])
                maxb_ap[c] = maxb

            def emit_stt_exp(c):
                """st = 10*st - maxb, then P = exp(st) in fp16 (batched)."""
                st, maxb = st_ap[c], maxb_ap[c]
                p_ap[c] = ppool.tile([128, KT, QCH], F16, tag="p", name="p")
                for kti in range(KT):
                    nc.vector.scalar_tensor_tensor(
                        st[:, kti, :],
                        st[:, kti, :],
                        10.0,
                        maxb[:],
                        op0=AL.mult,
                        op1=AL.subtract,
                    )
                    if kti % 4 == 3:
                        nc.scalar.activation(
                            p_ap[c][:, kti - 3 : kti + 1, :],
                            st[:, kti - 3 : kti + 1, :],
                            EXP,
                        )

            def emit_sum_recb(c):
                """Key-sums of P via ones-matmul, 1/sum, broadcast."""
                sum_ps = psR.tile([1, QCH], F32, tag="sum")
                for kti in range(KT):
                    nc.tensor.matmul(
                        sum_ps[:],
                        ones16[:],
                        p_ap[c][:, kti, :],
                        start=(kti == 0),
                        stop=(kti == KT - 1),
                    )
                recrow = treepool.tile([1, QCH], F32, tag="recrow")
                nc.vector.reciprocal(recrow[:], sum_ps[:])
                recb_ps = psX.tile([128, QCH], F32, tag="bcast")
                nc.tensor.matmul(
                    recb_ps[:], one32[:], recrow[:], start=True, stop=True
                )
                recb = auxpool.tile([128, QCH], F32, tag="recb")
                nc.vector.tensor_copy(recb[:], recb_ps[:])
                return recb

            def emit_pv(c, recb):
                """O^T[d, q] = V^T P scaled by 1/sum, written out as fp16."""
                q0 = QCH * c
                for dti in range(DT):
                    d0 = 128 * dti
                    ops = psO.tile([128, QCH], F32, tag="psO")
                    for kti in range(KT):
                        nc.tensor.matmul(
                            ops[:],
                            vf[:, kti, d0 : d0 + 128],
                            p_ap[c][:, kti, :],
                            start=(kti == 0),
                            stop=(kti == KT - 1),
                        )
                    osb = outpool.tile([128, QCH], F16, tag="osb")
                    nc.vector.scalar_tensor_tensor(
                        osb[:], ops[:], 1.0, recb[:], op0=AL.mult, op1=AL.mult
                    )
                    nc.sync.dma_start(ot_r[:, dti, q0 : q0 + QCH], osb[:])

            emit_qk_half(0, 0)
            emit_qk_half(0, 1)
            emit_qk_half(1, 0)
            for c in range(NCH):
                emit_tree_finish(c)
                emit_maxb(c)
                emit_stt_exp(c)
                if c + 1 < NCH:
                    emit_qk_half(c + 1, 1)
                recb = emit_sum_recb(c)
                if c + 2 < NCH:
                    emit_qk_half(c + 2, 0)
                emit_pv(c, recb)

    nc.compile()
    _BUILT["nc"] = nc
    return nc


def _tile_rows(a, blocks):
    """[D_in, cols] -> [blocks, 128, DT, cols/blocks]: row d, block j lands
    at [j, d % 128, d // 128, :]."""
    cols = a.shape[1] // blocks
    out = np.empty((blocks, 128, DT, cols), a.dtype)
    t = a.reshape(DT, 128, a.shape[1]).transpose(1, 0, 2)
    for j in range(blocks):
        out[j] = t[:, :, j * cols : (j + 1) * cols]
    return np.ascontiguousarray(out)


def _prep_inputs(x, q_w, k_w, v_w):
    a_full = (q_w.T.astype(np.float64) @ k_w.astype(np.float64)).astype(
        np.float32
    )
    am = _tile_rows(a_full.astype(np.float16), ET)
    wv = _tile_rows(v_w.T.astype(np.float16), 2)

    xk_b = []
    for b in range(B):
        xk_b.append(_tile_rows(np.asarray(x[b]).T.astype(np.float16), 4))

    in_maps = []
    for core in range(NCORES):
        b, h = divmod(core, 2)
        xq = _tile_rows(
            np.asarray(x[b, NQ * h : NQ * (h + 1)]).T.astype(np.float16), 2
        )
        in_maps.append({"xq": xq, "xk": xk_b[b], "am": am, "wv": wv})
    return in_maps


def run(x, q_w, k_w, v_w, trace=False):
    from concourse.bass_utils import run_bass_kernel_spmd

    nc = _build()
    in_maps = _prep_inputs(x, q_w, k_w, v_w)
    res = run_bass_kernel_spmd(nc, in_maps, list(range(NCORES)), trace=trace)
    out = np.empty((B, SEQ, D), np.float32)
    for core in range(NCORES):
        b, h = divmod(core, 2)
        out[b, NQ * h : NQ * (h + 1)] = res.results[core]["ot"].T.astype(np.float32)
    return out, res


def kernel(x, q_w, k_w, v_w):
    x = np.asarray(x, np.float32)
    q_w = np.asarray(q_w, np.float32)
    k_w = np.asarray(k_w, np.float32)
    v_w = np.asarray(v_w, np.float32)
    out, _ = run(x, q_w, k_w, v_w, trace=False)
    return out


# revision 10
# speedup vs baseline: 1.3075x; 1.0211x over previous
"""Single-head attention (B=4, N=2048, D=1024), scores scaled by 10.

Sharding: 8 cores = (batch, query-half). Core 2b+h owns queries
[1024h:1024(h+1)] of batch b. There is NO cross-core communication.

Algebraic restructuring (both sides of the softmax):
  scores = Q K^T = x (Wq^T Wk) x^T       -> A = Wq^T Wk on host,
                                             XA = x_own @ A on device,
                                             keys = raw full x
  out    = P V   = (P x_full) Wv^T       -> PX on device (PV-shaped),
                                             then one projection by Wv
The K and V projections and both pair-exchanges disappear; total
matmul work is unchanged (XA + QK + PX + PO = old Q/K/V + QK + PV) but
every collective, DRAM round-trip, and pair-skew dependency is gone.

Numerics: single-pass fp16 matmuls (fp32 PSUM accumulate); measured
end-to-end rel err ~4.6e-3 vs the fp32 reference (2e-2 gate). The
per-query max subtraction cancels exactly in the softmax
normalization, so max precision only guards overflow.

Schedule: inputs host-pre-tiled into contiguous >=1MB per-use DMAs
split across the two HWDGE rings; phase A is just XA (~28us); phase B
is a 2-deep software pipeline per 256-query chunk:
QK -> max-tree (interleaved with PSUM copies) -> exp -> sums -> PX ->
PO, with the next chunks' QK matmuls emitted around each softmax so
the PE never waits on the DVE/ACT chain.
"""

import numpy as np

B, SEQ, D = 4, 2048, 1024
NQ = 1024          # queries per core
QCH = 256          # attention q-chunk
NCH = NQ // QCH
NCORES = 8
DT = D // 128      # 8 d-tiles
ET = D // 128      # 8 e-tiles
KT = SEQ // 128    # 16 k-tiles

_BUILT = {}


def _build():
    if "nc" in _BUILT:
        return _BUILT["nc"]
    from contextlib import ExitStack

    import concourse.bass as bass  # noqa: F401
    import concourse.mybir as mybir
    import concourse.tile as tile
    from concourse import bacc

    dt = mybir.dt
    F32, F16 = dt.float32, dt.float16
    AL = mybir.AluOpType
    EXP = mybir.ActivationFunctionType.Exp

    nc = bacc.Bacc("TRN2", target_bir_lowering=False, debug=False)

    # host-pre-tiled inputs: each leading index is one contiguous DMA
    xq_d = nc.dram_tensor("xq", [2, 128, DT, 512], F16, kind="ExternalInput")
    xk_d = nc.dram_tensor("xk", [4, 128, DT, 512], F16, kind="ExternalInput")
    xr_d = nc.dram_tensor("xr", [4, 128, 4, D], F16, kind="ExternalInput")
    am_d = nc.dram_tensor("am", [ET, 128, DT, 128], F16, kind="ExternalInput")
    wv_d = nc.dram_tensor("wv", [ET, 128, DT, 128], F16, kind="ExternalInput")
    ot_d = nc.dram_tensor("ot", [D, NQ], F16, kind="ExternalOutput")

    xq_r = xq_d.ap()
    xk_r = xk_d.ap()
    xr_r = xr_d.ap()
    am_r = am_d.ap()
    wv_r = wv_d.ap()
    ot_r = ot_d.ap().rearrange("(t p) q -> p t q", p=128)

    with tile.TileContext(nc) as tc, ExitStack() as ctx:
        big_pool = ctx.enter_context(tc.tile_pool(name="big", bufs=1))
        # XA^T [d-part, d-tile, own queries]
        xat = big_pool.tile([128, ET, NQ], F16, tag="xat")
        # full x^T (the key matrix), 4 column blocks of 512 keys
        xk_t = [
            big_pool.tile([128, DT, 512], F16, tag=f"xk{i}", name=f"xk{i}")
            for i in range(4)
        ]
        # full x row-tiled (for PX), 4 blocks of 4 k-tiles
        xr_t = [
            big_pool.tile([128, 4, D], F16, tag=f"xr{i}", name=f"xr{i}")
            for i in range(4)
        ]
        # Wv^T tiled for the output projection
        wv_t = [
            big_pool.tile([128, DT, 128], F16, tag=f"wv{e}", name=f"wv{e}")
            for e in range(ET)
        ]

        const_pool = ctx.enter_context(tc.tile_pool(name="const", bufs=1))
        ones16 = const_pool.tile([128, 1], F16, tag="ones16")
        ten32 = const_pool.tile([1, 128], F32, tag="ten32")
        one32 = const_pool.tile([1, 128], F32, tag="one32")
        nc.vector.memset(ones16[:], 1.0)
        nc.vector.memset(ten32[:], 10.0)
        nc.vector.memset(one32[:], 1.0)

        with (
            tc.tile_pool(name="xspan", bufs=1) as xspan,
            tc.tile_pool(name="wstr", bufs=1) as wpool,
            tc.tile_pool(name="psA", bufs=4, space="PSUM") as psA,
        ):
            xq_t = [
                xspan.tile([128, DT, 512], F16, tag=f"xq{i}", name=f"xq{i}")
                for i in range(2)
            ]
            am_t = [
                wpool.tile([128, DT, 128], F16, tag=f"am{e}", name=f"am{e}")
                for e in range(ET)
            ]
            # input DMAs up front, split across the two HWDGE rings (sync ->
            # qSPDynamicHW, scalar -> qActDynamicHW), ordered by first use
            nc.scalar.dma_start(xq_t[0][:], xq_r[0])
            nc.sync.dma_start(xq_t[1][:], xq_r[1])
            for e in range(ET):
                nc.scalar.dma_start(am_t[e][:], am_r[e])
            for i in range(4):
                nc.sync.dma_start(xk_t[i][:], xk_r[i])
            for i in range(4):
                nc.sync.dma_start(xr_t[i][:], xr_r[i])
            for e in range(ET):
                nc.scalar.dma_start(wv_t[e][:], wv_r[e])

            # ------------- Phase A: XA^T = A^T @ x_own^T -------------------
            for et in range(ET):
                for chn in range(2):
                    n0 = 512 * chn
                    ps = psA.tile([128, 512], F32, tag="psA")
                    for dti in range(DT):
                        nc.tensor.matmul(
                            ps[:],
                            am_t[et][:, dti, :],
                            xq_t[chn][:, dti, :],
                            start=(dti == 0),
                            stop=(dti == DT - 1),
                        )
                    nc.vector.tensor_copy(xat[:, et, n0 : n0 + 512], ps[:])

        # ---------------- Phase B: attention, q-chunked, pipelined --------
        with (
            tc.tile_pool(name="stp", bufs=3) as stpool,
            tc.tile_pool(name="pp", bufs=2) as ppool,
            tc.tile_pool(name="pxp", bufs=2) as pxpool,
            tc.tile_pool(name="tree", bufs=2) as treepool,
            tc.tile_pool(name="aux", bufs=2) as auxpool,
            tc.tile_pool(name="osb", bufs=3) as outpool,
            tc.tile_pool(name="psS", bufs=2, space="PSUM") as psS,
            tc.tile_pool(name="psPX", bufs=2, space="PSUM") as psPX,
            tc.tile_pool(name="psPO", bufs=2, space="PSUM") as psPO,
            tc.tile_pool(name="psX", bufs=1, space="PSUM") as psX,
            tc.tile_pool(name="psR", bufs=1, space="PSUM") as psR,
        ):
            st_ap = {}
            t8_ap = {}
            p_ap = {}
            maxb_ap = {}
            m1_ap = {}

            def emit_qk_half(c, half):
                """Scores for k-tiles [8h, 8h+8) of chunk c, with the DVE
                max-tree levels interleaved behind the PSUM copies."""
                q0 = QCH * c
                if half == 0:
                    st_ap[c] = stpool.tile([128, KT, QCH], F32, tag="st", name="st")
                    t8_ap[c] = treepool.tile([128, 8, QCH], F32, tag="t8", name="t8")
                st, t8 = st_ap[c], t8_ap[c]
                for kti in range(8 * half, 8 * half + 8):
                    k0 = 128 * (kti % 4)
                    ps = psS.tile([128, QCH], F32, tag="psS")
                    for dti in range(DT):
                        nc.tensor.matmul(
                            ps[:],
                            xk_t[kti // 4][:, dti, k0 : k0 + 128],
                            xat[:, dti, q0 : q0 + QCH],
                            start=(dti == 0),
                            stop=(dti == DT - 1),
                        )
                    nc.vector.tensor_copy(st[:, kti, :], ps[:])
                    # interleave the max tree: level-0 after each odd tile,
                    # higher levels as their inputs complete
                    if kti % 2 == 1:
                        j = kti // 2
                        nc.vector.tensor_max(
                            t8[:, j, :], st[:, kti - 1, :], st[:, kti, :]
                        )
                        if j % 2 == 1:
                            nc.vector.tensor_max(
                                t8[:, j - 1, :], t8[:, j - 1, :], t8[:, j, :]
                            )
                        if j == 3:
                            nc.vector.tensor_max(
                                t8[:, 0, :], t8[:, 0, :], t8[:, 2, :]
                            )
                        if j == 7:
                            nc.vector.tensor_max(
                                t8[:, 4, :], t8[:, 4, :], t8[:, 6, :]
                            )

            def emit_tree_finish(c):
                """Final fold 128 partitions -> one [1, QCH] max row."""
                t8 = t8_ap[c]
                nc.vector.tensor_max(t8[:, 0, :], t8[:, 0, :], t8[:, 4, :])
                fold4 = treepool.tile([32, 4, QCH], F32, tag="fold4")
                for a in range(4):
                    nc.sync.dma_start(
                        fold4[:, a, :], t8[32 * a : 32 * (a + 1), 0, :]
                    )
                nc.vector.tensor_max(fold4[:, 0, :], fold4[:, 0, :], fold4[:, 1, :])
                nc.vector.tensor_max(fold4[:, 2, :], fold4[:, 2, :], fold4[:, 3, :])
                nc.vector.tensor_max(fold4[:, 0, :], fold4[:, 0, :], fold4[:, 2, :])
                t32t = treepool.tile([32, QCH], F32, tag="t32t")
                nc.vector.transpose(t32t[:], fold4[:, 0, :])
                mx32 = treepool.tile([32, 32], F32, tag="mx32")
                nc.vector.memset(mx32[:], 0.0)
                nc.vector.reduce_max(
                    mx32[:, 0 : QCH // 32],
                    t32t[:].rearrange("p (j c) -> p j c", c=32),
                    axis=mybir.AxisListType.X,
                )
                mx32t = treepool.tile([32, 32], F32, tag="mx32t")
                nc.vector.transpose(mx32t[:], mx32[:])
                m1row = treepool.tile([1, QCH], F32, tag="m1row")
                nc.sync.dma_start(m1row[:], mx32t[0 : QCH // 32, :])
                m1_ap[c] = m1row

            def emit_maxb(c):
                """Broadcast 10*max across partitions via rank-1 matmul."""
                maxb_ps = psX.tile([128, QCH], F32, tag="bcast")
                nc.tensor.matmul(
                    maxb_ps[:], ten32[:], m1_ap[c][:], start=True, stop=True
                )
                maxb = auxpool.tile([128, QCH], F32, tag="maxb")
                nc.vector.tensor_copy(maxb[:], maxb_ps[:])
                maxb_ap[c] = maxb

            def emit_stt_exp(c):
                """st = 10*st - maxb, then P = exp(st) in fp16 (batched)."""
                st, maxb = st_ap[c], maxb_ap[c]
                p_ap[c] = ppool.tile([128, KT, QCH], F16, tag="p", name="p")
                for kti in range(KT):
                    nc.vector.scalar_tensor_tensor(
                        st[:, kti, :],
                        st[:, kti, :],
                        10.0,
                        maxb[:],
                        op0=AL.mult,
                        op1=AL.subtract,
                    )
                    if kti % 4 == 3:
                        nc.scalar.activation(
                            p_ap[c][:, kti - 3 : kti + 1, :],
                            st[:, kti - 3 : kti + 1, :],
                            EXP,
                        )

            def emit_sum_recb(c):
                """Key-sums of P via ones-matmul, 1/sum, broadcast."""
                sum_ps = psR.tile([1, QCH], F32, tag="sum")
                for kti in range(KT):
                    nc.tensor.matmul(
                        sum_ps[:],
                        ones16[:],
                        p_ap[c][:, kti, :],
                        start=(kti == 0),
                        stop=(kti == KT - 1),
                    )
                recrow = treepool.tile([1, QCH], F32, tag="recrow")
                nc.vector.reciprocal(recrow[:], sum_ps[:])
                recb_ps = psX.tile([128, QCH], F32, tag="bcast")
                nc.tensor.matmul(
                    recb_ps[:], one32[:], recrow[:], start=True, stop=True
                )
                recb = auxpool.tile([128, QCH], F32, tag="recb")
                nc.vector.tensor_copy(recb[:], recb_ps[:])
                return recb

            def emit_px_po(c, recb):
                """PX^T = x^T P^T, then O^T = Wv PX^T scaled by 1/sum."""
                q0 = QCH * c
                pxt = pxpool.tile([128, DT, QCH], F16, tag="pxt", name="pxt")
                for dti in range(DT):
                    d0 = 128 * dti
                    ps = psPX.tile([128, QCH], F32, tag="psPX")
                    for kti in range(KT):
                        nc.tensor.matmul(
                            ps[:],
                            xr_t[kti // 4][:, kti % 4, d0 : d0 + 128],
                            p_ap[c][:, kti, :],
                            start=(kti == 0),
                            stop=(kti == KT - 1),
                        )
                    nc.vector.tensor_copy(pxt[:, dti, :], ps[:])
                for et in range(ET):
                    ops = psPO.tile([128, QCH], F32, tag="psPO")
                    for dti in range(DT):
                        nc.tensor.matmul(
                            ops[:],
                            wv_t[et][:, dti, :],
                            pxt[:, dti, :],
                            start=(dti == 0),
                            stop=(dti == DT - 1),
                        )
                    osb = outpool.tile([128, QCH], F16, tag="osb")
                    nc.vector.scalar_tensor_tensor(
                        osb[:], ops[:], 1.0, recb[:], op0=AL.mult, op1=AL.mult
                    )
                    nc.sync.dma_start(ot_r[:, et, q0 : q0 + QCH], osb[:])

            emit_qk_half(0, 0)
            emit_qk_half(0, 1)
            emit_qk_half(1, 0)
            for c in range(NCH):
                emit_tree_finish(c)
                emit_maxb(c)
                emit_stt_exp(c)
                if c + 1 < NCH:
                    emit_qk_half(c + 1, 1)
                recb = emit_sum_recb(c)
                if c + 2 < NCH:
                    emit_qk_half(c + 2, 0)
                emit_px_po(c, recb)

    nc.compile()
    _BUILT["nc"] = nc
    return nc


def _tile_cols(a, blocks):
    """[D_in, cols] -> [blocks, 128, D_in//128, cols/blocks]: row d of
    block j lands at [j, d % 128, d // 128, :]."""
    cols = a.shape[1] // blocks
    t = a.reshape(-1, 128, a.shape[1]).transpose(1, 0, 2)
    out = np.empty((blocks, 128, t.shape[1], cols), a.dtype)
    for j in range(blocks):
        out[j] = t[:, :, j * cols : (j + 1) * cols]
    return np.ascontiguousarray(out)


def _tile_rowblocks(a, blocks):
    """[rows, cols] -> [blocks, 128, rows//128//blocks, cols]: row r lands
    at [rt // (rows//128//blocks), r % 128, rt % ..., :] with rt = r//128."""
    t = a.reshape(-1, 128, a.shape[1]).transpose(1, 0, 2)  # [128, RT, cols]
    rt = t.shape[1] // blocks
    out = np.empty((blocks, 128, rt, a.shape[1]), a.dtype)
    for j in range(blocks):
        out[j] = t[:, j * rt : (j + 1) * rt, :]
    return np.ascontiguousarray(out)


def _prep_inputs(x, q_w, k_w, v_w):
    a_full = (q_w.T.astype(np.float64) @ k_w.astype(np.float64)).astype(
        np.float32
    )
    am = _tile_cols(a_full.astype(np.float16), ET)
    wv = _tile_cols(v_w.T.astype(np.float16), ET)

    xk_b, xr_b = [], []
    for b in range(B):
        xb = np.asarray(x[b]).astype(np.float16)
        xk_b.append(_tile_cols(np.ascontiguousarray(xb.T), 4))
        xr_b.append(_tile_rowblocks(xb, 4))

    in_maps = []
    for core in range(NCORES):
        b, h = divmod(core, 2)
        xq = _tile_cols(
            np.ascontiguousarray(np.asarray(x[b, NQ * h : NQ * (h + 1)]).T).astype(
                np.float16
            ),
            2,
        )
        in_maps.append(
            {"xq": xq, "xk": xk_b[b], "xr": xr_b[b], "am": am, "wv": wv}
        )
    return in_maps


def run(x, q_w, k_w, v_w, trace=False):
    from concourse.bass_utils import run_bass_kernel_spmd

    nc = _build()
    in_maps = _prep_inputs(x, q_w, k_w, v_w)
    res = run_bass_kernel_spmd(nc, in_maps, list(range(NCORES)), trace=trace)
    out = np.empty((B, SEQ, D), np.float32)
    for core in range(NCORES):
        b, h = divmod(core, 2)
        out[b, NQ * h : NQ * (h + 1)] = res.results[core]["ot"].T.astype(np.float32)
    return out, res


def kernel(x, q_w, k_w, v_w):
    x = np.asarray(x, np.float32)
    q_w = np.asarray(q_w, np.float32)
    k_w = np.asarray(k_w, np.float32)
    v_w = np.asarray(v_w, np.float32)
    out, _ = run(x, q_w, k_w, v_w, trace=False)
    return out
